# revision 16
# baseline (speedup 1.0000x reference)
"""Trainium2 Bass kernel for nn_ExpandedResolventFMNet.

Mathematical reformulation (validated in fp64 against the jax reference):

The reference builds kron(A.T, My) [8192x4096], its Gram [4096^2], resolvent
kron masks, and solves a dense 4096x4096 system.  All of that collapses to a
64x64 matrix equation solved by preconditioned CG (see kernel_v1 docstring
for the derivation):

  M'(Y) = G Y S~ + sum_d DdT * (G (DdT * Y)) = R~^T     (* = Hadamard)
  G     = My^T My,  S~ = Mx^T (A A^T) Mx,  A = Px fx
  R~^T  = My^T Bc A^T Mx,  Bc = My (Py fy)
  DdT   = resolvent-mask difference matrices (64x64), sqrt(LMBDA) folded in
  C     = Y Mx^T

Design (measured at ~88us vs the 167-207us v1 baseline): NO collectives.
On this runtime every collective pays a cross-core launch-skew barrier of
40-65us plus a ~10-14us AllReduce floor, which dominated the v1 timeline.
Instead every core redundantly computes the full pipeline from full inputs:

  - All matmuls run in fp16 (1 cycle/row vs fp32's 4 with double
    LDWEIGHTS; ~0.25us vs ~1.3us per 64x64 matmul).  A numpy fp16-rounding
    model of the whole solve tracks hardware rel-err within ~15% and puts
    this config (pcg=6, newton_s=7, newton_g=4) at ~2.2e-3; hardware
    measures ~4.0e-3 vs the 2e-2 gate (5x margin).
  - Big inputs are pre-cast to fp16 on the host (half the DMA bytes) and
    spread across three DMA queues (sync + scalar HWDGE, gpsimd SWDGE),
    x-side pieces first with tiny barrier-DMAs delaying the y-side, so the
    x-only chain (projection -> S~ -> Newton-S) starts while y streams.
    Per-core aggregate DMA tops out ~150 GB/s, making the ~25us input
    stream the hard floor of the timeline.
  - PCG dot products use a ones-matrix matmul for the cross-partition
    reduce+broadcast instead of gpsimd.partition_all_reduce.
  - Newton-Schulz (fp16) builds G^-1 and S~^-1 for the kron preconditioner;
    extra Newton steps are cheaper than PCG iterations, so the config
    trades 7 Newton-S steps for only 6 PCG iterations.
  - RHS is computed transpose-free as G (By A^T) Mx via
    (A By^T) -> By A^T Mx -> G(...), saving two matmul+cast links.
  - Engine queues are in-order, so emission order is scheduling: Newton-S
    (x-only) is emitted before the y-projection/RHS chain; op-chain casts
    run on the vector engine (idle there), PCG casts on scalar.
"""

import os

import numpy as np

import concourse.bacc as bacc
import concourse.mybir as mybir
from concourse.bass_utils import run_bass_kernel_spmd
from concourse.masks import make_identity
from concourse.tile import TileContext

F32 = mybir.dt.float32
F16 = mybir.dt.float16
K = 64          # spectral basis size
C = 128         # feature channels
V = 5000        # vertices
CHUNK = 125     # v-contraction tile (partition dim)
NCH = V // CHUNK            # 40 chunks, full V on every core
HALF = NCH // 2             # 20 chunks per DMA piece
N_CORES = int(os.environ.get("KCORES", "8"))
N_ITERS = 6
NEWTON_STEPS_S = 7
NEWTON_STEPS_G = 4
SQRT_LMBDA = 10.0

SHARD = False   # kept for test.py compat; ignored (always full-replication)

_PROGRAM_CACHE = {}


def build_program(shard: bool = False):
    nc = bacc.Bacc("TRN2", num_devices=N_CORES)

    # fp16 big inputs, host-prepared in [CHUNK, n, c] chunk-major layout,
    # split into two pieces each for DMA-queue parallelism
    QTR = NCH // 4
    fxq_d = [nc.dram_tensor(f"fxq{i}", [CHUNK, QTR * C], F16,
                            kind="ExternalInput") for i in range(4)]
    fyq_d = [nc.dram_tensor(f"fyq{i}", [CHUNK, QTR * C], F16,
                            kind="ExternalInput") for i in range(4)]
    px0_d = nc.dram_tensor("px0", [CHUNK, HALF * K], F16, kind="ExternalInput")
    px1_d = nc.dram_tensor("px1", [CHUNK, HALF * K], F16, kind="ExternalInput")
    py0_d = nc.dram_tensor("py0", [CHUNK, HALF * K], F16, kind="ExternalInput")
    py1_d = nc.dram_tensor("py1", [CHUNK, HALF * K], F16, kind="ExternalInput")
    # packed small fp16 matrices: [mx | my | mxT | myT] along free dim
    m4_d = nc.dram_tensor("m4", [K, 4 * K], F16, kind="ExternalInput")
    ev_d = nc.dram_tensor("ev", [1, 2 * K], F32, kind="ExternalInput")
    out_d = nc.dram_tensor("out", [K, K], F32, kind="ExternalOutput")
    barr_d = nc.dram_tensor("barr", [3, K], F16)

    with TileContext(nc) as tc:
        with (
            tc.tile_pool(name="big", bufs=1) as bp,
            tc.tile_pool(name="persist", bufs=1) as sp,
            tc.tile_pool(name="work", bufs=2) as wp,
            tc.tile_pool(name="pacc", bufs=1, space="PSUM") as pacc,
            tc.tile_pool(name="psum", bufs=2, space="PSUM") as pp,
        ):
            _ps_state = {"i": 0}

            def ps_tile(shape, dtype=F32):
                i = _ps_state["i"]
                _ps_state["i"] += 1
                return pp.tile(shape, dtype, tag=f"ps{i % 3}", name=f"pst{i}")

            # ---------------- DMA: small first, then big on 3 queues -------
            m4_s = sp.tile([K, 4 * K], F16)
            ev_t = sp.tile([1, 2 * K], F32)
            mxh = m4_s[:, 0:K]
            myh = m4_s[:, K:2 * K]
            mxTh = m4_s[:, 2 * K:3 * K]
            myTh = m4_s[:, 3 * K:4 * K]

            fxq_t = [bp.tile([CHUNK, QTR * C], F16, name=f"fxq{i}")
                     for i in range(4)]
            fyq_t = [bp.tile([CHUNK, QTR * C], F16, name=f"fyq{i}")
                     for i in range(4)]
            px0_t = bp.tile([CHUNK, HALF * K], F16)
            px1_t = bp.tile([CHUNK, HALF * K], F16)
            py0_t = bp.tile([CHUNK, HALF * K], F16)
            py1_t = bp.tile([CHUNK, HALF * K], F16)
            # All queued transfers share the per-core DMA bandwidth, and
            # the gpsimd SWDGE path is ~2x slower per byte than HWDGE, so
            # every x piece goes on the two HWDGE queues (sync + scalar),
            # followed by the small m4/ev loads (only needed mid-window),
            # then a tiny barrier DMA that reads an x tile (stalling the
            # engine until x has landed) before the y-side transfers.
            # gpsimd carries only the late-needed py pieces, also barriered.
            nc.sync.dma_start(px0_t, px0_d[:, :])
            nc.sync.dma_start(fxq_t[0], fxq_d[0][:, :])
            nc.sync.dma_start(fxq_t[1], fxq_d[1][:, :])
            nc.scalar.dma_start(px1_t, px1_d[:, :])
            nc.scalar.dma_start(fxq_t[2], fxq_d[2][:, :])
            nc.scalar.dma_start(fxq_t[3], fxq_d[3][:, :])
            nc.sync.dma_start(m4_s, m4_d[:, :])
            nc.sync.dma_start(ev_t, ev_d[:, :])
            nc.sync.dma_start(barr_d[0:1, :], fxq_t[1][0:1, 0:K])
            nc.scalar.dma_start(barr_d[1:2, :], fxq_t[3][0:1, 0:K])
            nc.gpsimd.dma_start(barr_d[2:3, :], fxq_t[2][0:1, 0:K])
            nc.sync.dma_start(fyq_t[0], fyq_d[0][:, :])
            nc.sync.dma_start(fyq_t[1], fyq_d[1][:, :])
            nc.scalar.dma_start(fyq_t[2], fyq_d[2][:, :])
            nc.scalar.dma_start(fyq_t[3], fyq_d[3][:, :])
            nc.gpsimd.dma_start(py0_t, py0_d[:, :])
            nc.gpsimd.dma_start(py1_t, py1_d[:, :])

            # ---------------- constants ------------------------------------
            id16 = sp.tile([K, K], F16)
            make_identity(nc, id16)
            id32 = sp.tile([K, K], F32)
            make_identity(nc, id32)
            ones16 = sp.tile([K, K], F16)
            nc.vector.memset(ones16, 1.0)
            ones_row = sp.tile([1, K], F16)
            nc.vector.memset(ones_row, 1.0)

            # ---------------- G = My^T My (early, hidden under DMA) --------
            g_ps = ps_tile([K, K])
            nc.tensor.matmul(g_ps, myh, myh)
            gh = sp.tile([K, K], F16)
            nc.vector.tensor_copy(gh, g_ps)

            # ---------------- resolvent masks ------------------------------
            # ev = [ex | ey]; t = ev/max(ev); im = sqrt(L)/(1+t);
            # re = sqrt(L)*sqrt(t)/(1+t)
            evmax = sp.tile([1, 1], F32)
            nc.vector.tensor_reduce(evmax, ev_t, mybir.AxisListType.X,
                                    mybir.AluOpType.max)
            evrec = sp.tile([1, 1], F32)
            nc.vector.reciprocal(evrec, evmax)
            t_t = sp.tile([1, 2 * K], F32)
            nc.vector.tensor_scalar_mul(t_t, ev_t, evrec)
            tp1 = sp.tile([1, 2 * K], F32)
            nc.vector.tensor_scalar_add(tp1, t_t, 1.0)
            im_t = sp.tile([1, 2 * K], F32)
            nc.vector.reciprocal(im_t, tp1)
            sq_t = sp.tile([1, 2 * K], F32)
            nc.scalar.sqrt(sq_t, t_t)
            re_t = sp.tile([1, 2 * K], F32)
            nc.vector.tensor_mul(re_t, sq_t, im_t)
            # fp16 copies (scaled by sqrt(LMBDA)) for the broadcast matmuls
            reh = sp.tile([1, 2 * K], F16)
            nc.vector.tensor_scalar_mul(reh, re_t, SQRT_LMBDA)
            imh = sp.tile([1, 2 * K], F16)
            nc.vector.tensor_scalar_mul(imh, im_t, SQRT_LMBDA)

            # D1T[a,i] = re2[a] - re1[i]; D2T likewise from im.
            # d12 = [D1T | D2T] in fp32 for the Hadamard ops.
            d12 = sp.tile([K, 2 * K], F32)
            for idx, src in enumerate((reh, imh)):
                pa = ps_tile([K, K])
                nc.tensor.matmul(pa, src[0:1, K:2 * K], ones_row)  # v2[a]
                pb = ps_tile([K, K])
                nc.tensor.matmul(pb, ones_row, src[0:1, 0:K])      # v1[i]
                ta = wp.tile([K, K], F32, tag=f"dta{idx}", name=f"dta{idx}")
                nc.vector.tensor_copy(ta, pa)
                nc.vector.tensor_sub(d12[:, idx * K:(idx + 1) * K], ta, pb)

            # ---------------- Newton-Schulz inverse (fp16) -----------------
            def newton_inverse(mat16, tag, steps, cast_engine):
                def cast(dst, src):
                    if cast_engine is nc.vector:
                        nc.vector.tensor_copy(dst, src)
                    else:
                        nc.scalar.copy(dst, src)

                # alpha = 1 / max_i sum_j |S_ij| via transpose + free reduce
                rs = wp.tile([K, 1], F32, tag=f"{tag}_rs", name=f"{tag}_rs")
                nc.vector.tensor_reduce(rs, mat16, mybir.AxisListType.X,
                                        mybir.AluOpType.add,
                                        apply_absolute_value=True)
                rsh = wp.tile([K, 1], F16, tag=f"{tag}_rsh", name=f"{tag}_rsh")
                cast(rsh, rs)
                rst_ps = ps_tile([1, K], F16)
                nc.tensor.transpose(rst_ps, rsh, id16)
                mx1 = wp.tile([1, 1], F32, tag=f"{tag}_mx1", name=f"{tag}_mx1")
                nc.vector.tensor_reduce(mx1, rst_ps, mybir.AxisListType.X,
                                        mybir.AluOpType.max)
                al1 = wp.tile([1, 1], F32, tag=f"{tag}_al1", name=f"{tag}_al1")
                nc.vector.reciprocal(al1, mx1)
                al1h = wp.tile([1, 1], F16, tag=f"{tag}_al1h",
                               name=f"{tag}_al1h")
                cast(al1h, al1)
                alp_ps = ps_tile([K, 1])
                nc.tensor.matmul(alp_ps, ones_row, al1h)   # broadcast [K,1]
                al = wp.tile([K, 1], F32, tag=f"{tag}_al", name=f"{tag}_al")
                nc.vector.tensor_copy(al, alp_ps)
                x16 = sp.tile([K, K], F16, tag=f"{tag}_x0", name=f"{tag}_x0")
                nc.vector.tensor_scalar_mul(x16, id16, al)
                for it in range(steps):
                    t1 = ps_tile([K, K])
                    nc.tensor.matmul(t1, mat16, x16)       # S X (S sym)
                    t1h = wp.tile([K, K], F16, tag=f"{tag}_t1h",
                                  name=f"{tag}_t1h")
                    cast(t1h, t1)
                    t2 = ps_tile([K, K])
                    nc.tensor.matmul(t2, x16, t1h)         # X (S X) (X sym)
                    xn = sp.tile([K, K], F16, tag=f"{tag}_x{it + 1}",
                                 name=f"{tag}_x{it + 1}")
                    nc.vector.scalar_tensor_tensor(
                        xn, x16, 2.0, t2,
                        op0=mybir.AluOpType.mult,
                        op1=mybir.AluOpType.subtract)
                    x16 = xn
                return x16

            # G-side Newton runs during the big DMAs (vector engine casts)
            gih = newton_inverse(gh, "gi", NEWTON_STEPS_G, nc.vector)

            # ---------------- projections: A^T = fx^T px, By^T = fy^T py ---
            at_p = pacc.tile([C, K], F32)
            byt_p = pacc.tile([C, K], F32)
            for n in range(NCH):
                fq, floc = divmod(n, QTR)
                piece, loc = divmod(n, HALF)
                pxt = (px0_t, px1_t)[piece]
                nc.tensor.matmul(at_p, fxq_t[fq][:, floc * C:(floc + 1) * C],
                                 pxt[:, loc * K:(loc + 1) * K],
                                 start=(n == 0), stop=(n == NCH - 1))
            ath = sp.tile([C, K], F16)
            nc.vector.tensor_copy(ath, at_p)

            # ---------------- S~ = Mx^T (A A^T) Mx -------------------------
            sa_ps = ps_tile([K, K])
            nc.tensor.matmul(sa_ps, ath, ath)              # A A^T
            sah = sp.tile([K, K], F16)
            nc.vector.tensor_copy(sah, sa_ps)
            m1_ps = ps_tile([K, K])
            nc.tensor.matmul(m1_ps, sah, mxh)              # S_A Mx (sym)
            m1h = sp.tile([K, K], F16)
            nc.vector.tensor_copy(m1h, m1_ps)
            st_ps = ps_tile([K, K])
            nc.tensor.matmul(st_ps, mxh, m1h)              # Mx^T (S_A Mx)
            sth = sp.tile([K, K], F16)
            nc.vector.tensor_copy(sth, st_ps)

            sih = newton_inverse(sth, "si", NEWTON_STEPS_S, nc.vector)

            # y-projection + RHS chain, emitted after Newton-S: the tensor
            # queue is in-order and y-data lands while Newton-S runs on
            # x-side data.  RHS' = My^T Bc A^T Mx = G By A^T Mx, computed
            # transpose-free as mm chains (A By^T) -> By A^T Mx -> G(...).
            for n in range(NCH):
                fq, floc = divmod(n, QTR)
                piece, loc = divmod(n, HALF)
                pyt = (py0_t, py1_t)[piece]
                nc.tensor.matmul(byt_p, fyq_t[fq][:, floc * C:(floc + 1) * C],
                                 pyt[:, loc * K:(loc + 1) * K],
                                 start=(n == 0), stop=(n == NCH - 1))
            byth = sp.tile([C, K], F16)
            nc.vector.tensor_copy(byth, byt_p)
            abyt_ps = ps_tile([K, K])
            nc.tensor.matmul(abyt_ps, ath, byth)           # A By^T
            abyth = sp.tile([K, K], F16)
            nc.vector.tensor_copy(abyth, abyt_ps)
            f2_ps = ps_tile([K, K])
            nc.tensor.matmul(f2_ps, abyth, mxh)            # By A^T Mx
            f2h = sp.tile([K, K], F16)
            nc.vector.tensor_copy(f2h, f2_ps)
            r0_ps = ps_tile([K, K])
            nc.tensor.matmul(r0_ps, gh, f2h)               # G By A^T Mx
            r_s = sp.tile([K, K], F32)
            nc.vector.tensor_copy(r_s, r0_ps)

            # ---------------- PCG (pipelined, fp16 matmuls) ----------------
            y_s = sp.tile([K, K], F32)
            nc.vector.memset(y_s, 0.0)
            p_s = sp.tile([K, K], F32)
            q_s = sp.tile([K, K], F32)
            s_s = sp.tile([K, K], F32)
            z_s = sp.tile([K, K], F32)
            u16 = sp.tile([K, 2 * K], F16)   # stacked [D1T*z | D2T*z]

            def precond_ps(x_f32, tag):
                """P^-1 x = Gi x Si in PSUM via (Gi x)^T = mm(x, Gi)."""
                xh = wp.tile([K, K], F16, tag=f"{tag}_xh", name=f"{tag}_xh")
                nc.scalar.copy(xh, x_f32)
                ut_ps = ps_tile([K, K])
                nc.tensor.matmul(ut_ps, xh, gih)
                uth = wp.tile([K, K], F16, tag=f"{tag}_uth", name=f"{tag}_uth")
                nc.scalar.copy(uth, ut_ps)
                v_ps = ps_tile([K, K])
                nc.tensor.matmul(v_ps, uth, sih)
                return v_ps

            def matvec_z():
                """w = M z into SBUF f32 (reads z_s)."""
                zh = wp.tile([K, K], F16, tag="mv_zh", name="mv_zh")
                nc.scalar.copy(zh, z_s)
                nc.vector.tensor_mul(u16[:, 0:K], d12[:, 0:K], z_s)
                nc.vector.tensor_mul(u16[:, K:2 * K], d12[:, K:2 * K], z_s)
                gzt_ps = ps_tile([K, K])
                nc.tensor.matmul(gzt_ps, zh, gh)           # (G z)^T
                gzth = wp.tile([K, K], F16, tag="mv_gzth", name="mv_gzth")
                nc.scalar.copy(gzth, gzt_ps)
                t2_ps = ps_tile([K, K])
                nc.tensor.matmul(t2_ps, gzth, sth)         # (G z) S~
                gu_ps = ps_tile([K, 2 * K])
                nc.tensor.matmul(gu_ps, gh, u16)           # G [u1|u2]
                mm_s = wp.tile([K, 2 * K], F32, tag="mv_mm", name="mv_mm")
                nc.vector.tensor_mul(mm_s, d12, gu_ps)     # masked halves
                a1_s = wp.tile([K, K], F32, tag="mv_a1", name="mv_a1")
                nc.vector.tensor_add(a1_s, mm_s[:, 0:K], t2_ps)
                w_s = wp.tile([K, K], F32, tag="mv_w", name="mv_w")
                nc.vector.tensor_add(w_s, a1_s, mm_s[:, K:2 * K])
                return w_s

            def dot_ps(a_ap, b_ap, tag):
                """<a,b> summed over all elements, broadcast as [K,1] PSUM."""
                prod = wp.tile([K, K], F32, tag="dot_dm", name="dot_dm")
                acc = wp.tile([K, 1], F32, tag=f"{tag}_acc", name=f"{tag}_acc")
                nc.vector.scalar_tensor_tensor(
                    prod, a_ap, 1.0, b_ap,
                    op0=mybir.AluOpType.bypass, op1=mybir.AluOpType.mult,
                    accum_out=acc)
                ach = wp.tile([K, 1], F16, tag=f"{tag}_ach", name=f"{tag}_ach")
                nc.scalar.copy(ach, acc)
                d_ps = ps_tile([K, 1])
                nc.tensor.matmul(d_ps, ones16, ach)        # sum + broadcast
                return d_ps

            # init: z = P^-1 r; p = z; w = Mz; q = w; s = P^-1 w
            z0_ps = precond_ps(r_s, "pcz")
            nc.vector.tensor_copy(z_s, z0_ps)
            nc.vector.tensor_copy(p_s, z0_ps)
            rz_ps = dot_ps(r_s, z_s, "rz")
            rzs = wp.tile([K, 1], F32, tag="rzs", name="rzs")
            nc.scalar.copy(rzs, rz_ps)
            rzrec = wp.tile([K, 1], F32, tag="rzrec", name="rzrec")
            nc.vector.reciprocal(rzrec, rz_ps)
            w_s = matvec_z()
            nc.vector.tensor_copy(q_s, w_s)
            v_ps = precond_ps(w_s, "pcv")
            nc.vector.tensor_copy(s_s, v_ps)

            for it in range(N_ITERS):
                pq_ps = dot_ps(p_s, q_s, "pq")
                pqr = wp.tile([K, 1], F32, tag="pqr", name="pqr")
                nc.vector.reciprocal(pqr, pq_ps)
                al = wp.tile([K, 1], F32, tag="al", name="al")
                nc.vector.tensor_mul(al, rzs, pqr)
                if it < N_ITERS - 1:
                    an = wp.tile([K, 1], F32, tag="an", name="an")
                    nc.scalar.mul(an, al, -1.0)
                    nc.vector.scalar_tensor_tensor(
                        r_s, q_s, an, r_s,
                        op0=mybir.AluOpType.mult, op1=mybir.AluOpType.add)
                    nc.vector.scalar_tensor_tensor(
                        z_s, s_s, an, z_s,
                        op0=mybir.AluOpType.mult, op1=mybir.AluOpType.add)
                nc.vector.scalar_tensor_tensor(
                    y_s, p_s, al, y_s,
                    op0=mybir.AluOpType.mult, op1=mybir.AluOpType.add)

                if it == N_ITERS - 1:
                    break

                rznew_ps = dot_ps(r_s, z_s, "rz")
                w_s = matvec_z()
                if it < N_ITERS - 2:
                    v_ps = precond_ps(w_s, "pcv")
                bt = wp.tile([K, 1], F32, tag="bt", name="bt")
                nc.vector.tensor_mul(bt, rznew_ps, rzrec)
                nc.vector.scalar_tensor_tensor(
                    p_s, p_s, bt, z_s,
                    op0=mybir.AluOpType.mult, op1=mybir.AluOpType.add)
                nc.vector.scalar_tensor_tensor(
                    q_s, q_s, bt, w_s,
                    op0=mybir.AluOpType.mult, op1=mybir.AluOpType.add)
                if it < N_ITERS - 2:
                    nc.vector.scalar_tensor_tensor(
                        s_s, s_s, bt, v_ps,
                        op0=mybir.AluOpType.mult, op1=mybir.AluOpType.add)
                rzs = wp.tile([K, 1], F32, tag="rzs", name="rzs")
                nc.scalar.copy(rzs, rznew_ps)
                rzrec = wp.tile([K, 1], F32, tag="rzrec", name="rzrec")
                nc.vector.reciprocal(rzrec, rznew_ps)

            # ---------------- output: C = Y Mx^T ----------------
            yt_ps = ps_tile([K, K])
            nc.tensor.transpose(yt_ps, y_s, id32)
            yth = wp.tile([K, K], F16, tag="yth", name="yth")
            nc.scalar.copy(yth, yt_ps)
            c_ps = ps_tile([K, K])
            nc.tensor.matmul(c_ps, yth, mxTh)              # Y Mx^T
            c_s = wp.tile([K, K], F32, tag="c_s", name="c_s")
            nc.vector.tensor_copy(c_s, c_ps)
            nc.sync.dma_start(out_d[:, :], c_s)

    nc.finalize()
    return nc


def get_program(shard: bool = False):
    if shard not in _PROGRAM_CACHE:
        _PROGRAM_CACHE[shard] = build_program(shard)
    return _PROGRAM_CACHE[shard]


def _chunk_major(a, width):
    """[V, width] -> [CHUNK, NCH*width] fp16 halves in chunk-major layout."""
    a16 = np.asarray(a, np.float16).reshape(NCH, CHUNK, width)
    a16 = a16.transpose(1, 0, 2)  # [CHUNK, NCH, width]
    h0 = np.ascontiguousarray(a16[:, :HALF].reshape(CHUNK, HALF * width))
    h1 = np.ascontiguousarray(a16[:, HALF:].reshape(CHUNK, HALF * width))
    return h0, h1


def make_in_maps(inputs, shard: bool = False):
    fx = np.asarray(inputs["feat_x"], np.float32)[0]
    fy = np.asarray(inputs["feat_y"], np.float32)[0]
    pxT = np.asarray(inputs["evecs_trans_x"], np.float32)[0].T
    pyT = np.asarray(inputs["evecs_trans_y"], np.float32)[0].T
    mx = np.asarray(inputs["sqrtMk_x"], np.float32)[0]
    my = np.asarray(inputs["sqrtMk_y"], np.float32)[0]
    ev = np.ascontiguousarray(np.concatenate([
        np.asarray(inputs["evals_x"], np.float32)[0],
        np.asarray(inputs["evals_y"], np.float32)[0],
    ])[None, :])

    px0, px1 = _chunk_major(pxT, K)
    py0, py1 = _chunk_major(pyT, K)
    QTR = NCH // 4
    fx16 = np.asarray(fx, np.float16).reshape(NCH, CHUNK, C).transpose(1, 0, 2)
    fxq = [np.ascontiguousarray(
        fx16[:, i * QTR:(i + 1) * QTR].reshape(CHUNK, QTR * C))
        for i in range(4)]
    fy16 = np.asarray(fy, np.float16).reshape(NCH, CHUNK, C).transpose(1, 0, 2)
    fyq = [np.ascontiguousarray(
        fy16[:, i * QTR:(i + 1) * QTR].reshape(CHUNK, QTR * C))
        for i in range(4)]
    m4 = np.ascontiguousarray(np.concatenate(
        [mx, my, mx.T, my.T], axis=1).astype(np.float16))

    m = {
        "fxq0": fxq[0], "fxq1": fxq[1], "fxq2": fxq[2], "fxq3": fxq[3],
        "fyq0": fyq[0], "fyq1": fyq[1], "fyq2": fyq[2], "fyq3": fyq[3],
        "px0": px0, "px1": px1, "py0": py0, "py1": py1,
        "m4": m4, "ev": ev,
    }
    return [m for _ in range(N_CORES)]


def kernel(**inputs) -> np.ndarray:
    nc = get_program(SHARD)
    in_maps = make_in_maps(inputs, SHARD)
    res = run_bass_kernel_spmd(nc, in_maps, core_ids=list(range(N_CORES)))
    out = np.asarray(res.results[0]["out"], dtype=np.float32)
    return out[None]
